# revision 12
# baseline (speedup 1.0000x reference)
"""Trainium2 Bass kernel for nn_Equivariant_257698037971.

Computes out = relu(x @ lam - (sum_m x) @ gam) for x [B, M, F] = [8192, 512, 64],
lam/gam [F, O] = [64, 128], out [B, M, O] fp32.

Strategy (data-parallel over batch, 8 NeuronCores, no collectives):
  - Default layout is "pair" (see build_nc_pair): 2 batches share the 128
    partitions (p = r*64 + m//8) so per-partition HBM chunks are 2 KiB on
    load / 4 KiB on store. KERNEL_LAYOUT=quad selects the older layout
    below (1 KiB / 2 KiB chunks); both measure ~4.14 ms end-to-end, of
    which ~2.5 ms is flat axon-proxy per-call dispatch overhead and
    ~1.2-1.4 ms device time vs a 1.125 ms HBM roofline.
  - quad: each core gets 1024 batches. Per batch (x_b is [512, 64]):
    * x loaded in 2 MiB groups of 16 batches, cast fp32->bf16 in the DMA
      (SWDGE cast). SBUF layout per batch: [128 part, 256] where partition p
      holds rows 4p..4p+3 (fully contiguous HBM reads).
    * PE "transpose" via matmul with rhs = [I_128 | ones]: one [128,128]
      slice per instruction yields the transposed stack AND the per-column
      row-sum (pooling partial sums) in an extra 129th column.
    * xT copied PSUM->SBUF as bf16 (one [128, 258] copy on VectorE).
    * sbc [128, 128] = (scol0 + scol1) broadcast along free (one VectorE
      tensor_scalar reading the fp32 s-columns straight from PSUM).
    * output PSUM bank [128, 512]: one matmul (lhsT = sbc, rhs = -gam tiled
      4x) broadcasts -pooled into all 4 regions and opens the accumulation
      group; two zero-padded K=128 matmuls (rhs = [[lam;0]|[0;lam]])
      accumulate x@lam for all 4 row classes.
    * ReLU fused in one activation PSUM->SBUF (fp32), split 3:1
      between ScalarE and VectorE to balance engine load.
    * Stores batched as 2 MiB DMAs (8 batches), alternating between
      the SP (HWDGE) and gpsimd (SWDGE) queues to spread issue cost.
"""

import os
import sys
from contextlib import ExitStack

import numpy as np

sys.path.insert(0, "/opt/trn_rl_repo")

import concourse.bass as bass
import concourse.mybir as mybir
import concourse.tile as tile
from concourse.bass_utils import run_bass_kernel_spmd

B, M, F, O = 8192, 512, 64, 128
N_CORES = 8
G_IN = int(os.environ.get("KERNEL_G_IN", "16"))
G_OUT = int(os.environ.get("KERNEL_G_OUT", "8"))

_BF16 = mybir.dt.np(mybir.dt.bfloat16)

# Results of the last run (for test harness introspection).
LAST_RUN = {}


LAYOUT = os.environ.get("KERNEL_LAYOUT", "pair")


def build_nc(shard_b):
    if LAYOUT == "pair":
        return build_nc_pair(shard_b)
    dt = mybir.dt
    nc = bass.Bass(trn_type="TRN2")

    x_d = nc.dram_tensor("x", [shard_b, M, F], dt.float32, kind="ExternalInput")
    ione_d = nc.dram_tensor("ione", [128, 129], dt.bfloat16, kind="ExternalInput")
    lam_d = nc.dram_tensor("lampad", [128, 2 * O], dt.bfloat16, kind="ExternalInput")
    gam_d = nc.dram_tensor("gamneg4", [128, 4 * O], dt.bfloat16, kind="ExternalInput")
    ones_d = nc.dram_tensor("ones128", [128, 128], dt.bfloat16, kind="ExternalInput")
    out_d = nc.dram_tensor("out", [shard_b, M, O], dt.float32, kind="ExternalOutput")

    # x element (b, m, f), b = G_IN*g + r, m = 4p + q:
    #   x_view[g, p, r, 64q + f]; per-partition HBM reads are 1 KiB contiguous.
    x_view = x_d.rearrange("(g r) (p q) f -> g p r (q f)", r=G_IN, p=128, q=4)
    # out element (b, m, o), b = G_OUT*g + r, m = 4p + j:
    #   out_view[g, p, r, 128j + o]; per-partition HBM writes are 2 KiB contiguous.
    out_view = out_d.rearrange("(g r) (p q) o -> g p r (q o)", r=G_OUT, p=128, q=4)

    with ExitStack() as ctx:
        tc = ctx.enter_context(tile.TileContext(nc))

        cpool = ctx.enter_context(tc.tile_pool(name="consts", bufs=1))
        ione = cpool.tile([128, 129], dt.bfloat16, name="ione_sb")
        lam_s = cpool.tile([128, 2 * O], dt.bfloat16, name="lam_sb")
        gam_s = cpool.tile([128, 4 * O], dt.bfloat16, name="gam_sb")
        ones_s = cpool.tile([128, 128], dt.bfloat16, name="ones_sb")
        nc.sync.dma_start(out=ione[:], in_=ione_d[:])
        nc.sync.dma_start(out=lam_s[:], in_=lam_d[:])
        nc.sync.dma_start(out=gam_s[:], in_=gam_d[:])
        nc.sync.dma_start(out=ones_s[:], in_=ones_d[:])

        def _bufs(name, dflt):
            return int(os.environ.get(f"KERNEL_BUFS_{name}", str(dflt)))
        xpool = ctx.enter_context(tc.tile_pool(name="xin", bufs=_bufs("XIN", 3)))
        xtpool = ctx.enter_context(tc.tile_pool(name="xtsb", bufs=_bufs("XT", 3)))
        sbcpool = ctx.enter_context(tc.tile_pool(name="sbcsb", bufs=_bufs("SBC", 3)))
        opool = ctx.enter_context(tc.tile_pool(name="outsb", bufs=_bufs("OUT", 4)))
        tpsum = ctx.enter_context(tc.tile_pool(name="tpsum", bufs=_bufs("TP", 3), space="PSUM"))
        mpsum = ctx.enter_context(tc.tile_pool(name="mpsum", bufs=_bufs("MP", 4), space="PSUM"))

        load_mode = os.environ.get("KERNEL_LOAD", "swdge")
        xfpool = None
        if load_mode == "hwdge":
            xfpool = ctx.enter_context(
                tc.tile_pool(name="xinf", bufs=_bufs("XINF", 3))
            )

        out4 = None
        repeat = int(os.environ.get("KERNEL_REPEAT", "1"))
        for g in list(range(shard_b // G_IN)) * repeat:
            x8 = xpool.tile([128, G_IN, 256], dt.bfloat16, name="x8")
            if load_mode == "hwdge":
                # plain fp32 load on the ACT HWDGE ring, then DVE downcast
                x8f = xfpool.tile([128, G_IN, 256], dt.float32, name="x8f")
                nc.scalar.dma_start(out=x8f[:], in_=x_view[g])
                nc.vector.tensor_copy(x8[:], x8f[:])
            else:
                # fp32 -> bf16 cast happens inside the (SWDGE) DMA.
                nc.gpsimd.dma_start(out=x8[:], in_=x_view[g])
            for r in range(G_IN):
                b = g * G_IN + r
                ro = b % G_OUT
                if ro == 0:
                    out4 = opool.tile([128, G_OUT * 512], dt.float32, name="out4")

                # Transpose both [128, 128] column-slices of this batch, each
                # with an appended row-sum column (the ones column of ione).
                pt = tpsum.tile([128, 258], dt.float32, name="pt")
                nc.tensor.matmul(
                    pt[:, 0:129], lhsT=x8[:, r, 0:128], rhs=ione[:],
                    start=True, stop=True,
                )
                nc.tensor.matmul(
                    pt[:, 129:258], lhsT=x8[:, r, 128:256], rhs=ione[:],
                    start=True, stop=True,
                )
                xt = xtpool.tile([128, 258], dt.bfloat16, name="xt")
                nc.vector.tensor_copy(xt[:], pt[:])

                # sbc[q, i] = scol0[q] + scol1[q] for all i — the combined
                # per-batch column sums, broadcast along the free dim.
                sbc = sbcpool.tile([128, 128], dt.bfloat16, name="sbc")
                nc.vector.tensor_scalar(
                    sbc[:], ones_s[:], pt[:, 128:129], pt[:, 257:258],
                    mybir.AluOpType.mult, mybir.AluOpType.add,
                )

                # Output bank: one matmul broadcasts -pooled into all 4
                # regions (group opener), then 2 zero-padded K=128 main
                # matmuls accumulate x @ lam.
                pm = mpsum.tile([128, 4 * O], dt.float32, name="pm")
                nc.tensor.matmul(
                    pm[:], lhsT=sbc[:], rhs=gam_s[:],
                    start=True, stop=False, skip_group_check=True,
                )
                for a in range(2):
                    nc.tensor.matmul(
                        pm[:, 2 * O * a:2 * O * (a + 1)],
                        lhsT=xt[:, 129 * a:129 * a + 128],
                        rhs=lam_s[:],
                        start=False, stop=(a == 1), skip_group_check=True,
                    )

                if b % 4 == int(os.environ.get("KERNEL_RELU_ALT", "5")):
                    nc.vector.tensor_scalar(
                        out4[:, 512 * ro:512 * (ro + 1)], pm[:], 0.0, None,
                        mybir.AluOpType.max,
                    )
                else:
                    nc.scalar.activation(
                        out4[:, 512 * ro:512 * (ro + 1)], pm[:],
                        mybir.ActivationFunctionType.Relu,
                    )
                if ro == G_OUT - 1:
                    gg = b // G_OUT
                    ds = os.environ.get("KERNEL_DUAL_STORE", "pool")
                    if ds == "pool":
                        eng = nc.gpsimd if gg % 2 == 0 else nc.sync
                    elif ds == "1":
                        eng = nc.scalar if gg % 2 == 0 else nc.sync
                    else:
                        eng = nc.sync
                    eng.dma_start(
                        out=out_view[gg],
                        in_=out4[:].rearrange("p (r c) -> p r c", r=G_OUT),
                    )
    _split_multi_waits(nc)
    return nc


def build_nc_pair(shard_b):
    """Pair layout: 2 batches share the 128 partitions (p = r*64 + m//8),
    so every per-partition HBM chunk is 2 KiB on load and 4 KiB on store
    (vs 1 KiB / 2 KiB for the quad layout) — fewer SDMA descriptors per
    byte. Per pair: 4 transposes of [128,128] slices against
    ione2 = [I_128 | r0-ones | r1-ones] give xT for q-class pairs plus
    per-batch partial sums; 2 PSUM banks accumulate -pooled + x@lam for
    q-classes 0..3 / 4..7; ReLU into a [128, 1024]-per-pair out tile."""
    dt = mybir.dt
    nc = bass.Bass(trn_type="TRN2")
    TP = G_IN // 2   # pairs per input group
    TO = G_OUT // 2  # pairs per output group

    x_d = nc.dram_tensor("x", [shard_b, M, F], dt.float32, kind="ExternalInput")
    ione_d = nc.dram_tensor("ione", [128, 130], dt.bfloat16, kind="ExternalInput")
    lam_d = nc.dram_tensor("lampad", [128, 2 * O], dt.bfloat16, kind="ExternalInput")
    gam_d = nc.dram_tensor("gamneg4", [128, 4 * O], dt.bfloat16, kind="ExternalInput")
    ones_d = nc.dram_tensor("ones128", [128, 128], dt.bfloat16, kind="ExternalInput")
    out_d = nc.dram_tensor("out", [shard_b, M, O], dt.float32, kind="ExternalOutput")

    # x element (b, m, f), b = G_IN*g + 2t + r, m = 8*m8 + q:
    #   x_view[g, 64r + m8, t, 64q + f]; per-partition chunks are 2 KiB.
    x_view = x_d.rearrange("(g t r) (m8 q) f -> g (r m8) t (q f)", t=TP, r=2, q=8)
    # out element (b, m, o), b = G_OUT*gg + 2tt + r, m = 8*m8 + q:
    #   out_view[gg, 64r + m8, tt, 128q + o]; per-partition chunks are 4 KiB.
    out_view = out_d.rearrange(
        "(gg tt r) (m8 q) o -> gg (r m8) tt (q o)", tt=TO, r=2, q=8
    )

    with ExitStack() as ctx:
        tc = ctx.enter_context(tile.TileContext(nc))

        cpool = ctx.enter_context(tc.tile_pool(name="consts", bufs=1))
        ione = cpool.tile([128, 130], dt.bfloat16, name="ione_sb")
        lam_s = cpool.tile([128, 2 * O], dt.bfloat16, name="lam_sb")
        gam_s = cpool.tile([128, 4 * O], dt.bfloat16, name="gam_sb")
        ones_s = cpool.tile([128, 128], dt.bfloat16, name="ones_sb")
        nc.sync.dma_start(out=ione[:], in_=ione_d[:])
        nc.sync.dma_start(out=lam_s[:], in_=lam_d[:])
        nc.sync.dma_start(out=gam_s[:], in_=gam_d[:])
        nc.sync.dma_start(out=ones_s[:], in_=ones_d[:])

        def _bufs(name, dflt):
            return int(os.environ.get(f"KERNEL_BUFS_{name}", str(dflt)))
        xpool = ctx.enter_context(tc.tile_pool(name="xin", bufs=_bufs("XIN", 4)))
        xtpool = ctx.enter_context(tc.tile_pool(name="xtsb", bufs=_bufs("XT", 4)))
        sspool = ctx.enter_context(tc.tile_pool(name="sssb", bufs=_bufs("SS", 4)))
        sbcpool = ctx.enter_context(tc.tile_pool(name="sbcsb", bufs=_bufs("SBC", 3)))
        opool = ctx.enter_context(tc.tile_pool(name="outsb", bufs=_bufs("OUT", 6)))
        tpsum = ctx.enter_context(tc.tile_pool(name="tpsum", bufs=_bufs("TP", 2), space="PSUM"))
        mpsum = ctx.enter_context(tc.tile_pool(name="mpsum", bufs=_bufs("MP", 2), space="PSUM"))

        out4 = None
        repeat = int(os.environ.get("KERNEL_REPEAT", "1"))
        for g in list(range(shard_b // G_IN)) * repeat:
            x8 = xpool.tile([128, TP, 512], dt.bfloat16, name="x8")
            nc.gpsimd.dma_start(out=x8[:], in_=x_view[g])
            for t in range(TP):
                pr = g * TP + t
                tt = pr % TO
                if tt == 0:
                    out4 = opool.tile([128, TO * 1024], dt.float32, name="out4")

                # Transpose the 4 [128, 128] column-slices of this pair;
                # each yields 128 xT columns + 2 per-batch sum columns.
                pts = []
                for h in range(2):
                    pt = tpsum.tile([128, 260], dt.float32, name=f"pt{h}")
                    for j in range(2):
                        nc.tensor.matmul(
                            pt[:, 130 * j:130 * (j + 1)],
                            lhsT=x8[:, t, 128 * (2 * h + j):128 * (2 * h + j + 1)],
                            rhs=ione[:], start=True, stop=True,
                        )
                    pts.append(pt)
                xts = []
                for h in range(2):
                    xt = xtpool.tile([128, 260], dt.bfloat16, name=f"xt{h}")
                    nc.vector.tensor_copy(xt[:], pts[h][:])
                    xts.append(xt)

                # Per-batch partial sums: s01/s23[:, r] = the two q-class
                # sums of each transpose half; sbc halves broadcast
                # s01 + s23 along free for batch 0 (cols 0:64) / 1 (64:128).
                # (HW: only one tensor-op input may come from PSUM, so read
                # the bf16 sum columns from the xt SBUF copies instead.)
                s01 = sspool.tile([128, 2], dt.float32, name="s01")
                s23 = sspool.tile([128, 2], dt.float32, name="s23")
                nc.vector.tensor_tensor(
                    s01[:], xts[0][:, 128:130], xts[0][:, 258:260],
                    mybir.AluOpType.add,
                )
                nc.vector.tensor_tensor(
                    s23[:], xts[1][:, 128:130], xts[1][:, 258:260],
                    mybir.AluOpType.add,
                )
                sbc = sbcpool.tile([128, 128], dt.bfloat16, name="sbc")
                for r in range(2):
                    nc.vector.tensor_scalar(
                        sbc[:, 64 * r:64 * (r + 1)], ones_s[:, 0:64],
                        s01[:, r:r + 1], s23[:, r:r + 1],
                        mybir.AluOpType.mult, mybir.AluOpType.add,
                    )

                # 2 output banks: q-classes 0..3 (h=0) and 4..7 (h=1).
                for h in range(2):
                    pm = mpsum.tile([128, 4 * O], dt.float32, name=f"pm{h}")
                    nc.tensor.matmul(
                        pm[:], lhsT=sbc[:], rhs=gam_s[:],
                        start=True, stop=False, skip_group_check=True,
                    )
                    for a in range(2):
                        nc.tensor.matmul(
                            pm[:, 2 * O * a:2 * O * (a + 1)],
                            lhsT=xts[h][:, 130 * a:130 * a + 128],
                            rhs=lam_s[:],
                            start=False, stop=(a == 1), skip_group_check=True,
                        )
                    dst = out4[:, 1024 * tt + 512 * h:1024 * tt + 512 * (h + 1)]
                    if (2 * pr + h) % 4 == int(os.environ.get("KERNEL_RELU_ALT", "5")):
                        nc.vector.tensor_scalar(
                            dst, pm[:], 0.0, None, mybir.AluOpType.max,
                        )
                    else:
                        nc.scalar.activation(
                            dst, pm[:], mybir.ActivationFunctionType.Relu,
                        )

                if tt == TO - 1:
                    gg = pr // TO
                    ds = os.environ.get("KERNEL_DUAL_STORE", "pool")
                    if ds == "pool":
                        eng = nc.gpsimd if gg % 2 == 0 else nc.sync
                    elif ds == "1":
                        eng = nc.scalar if gg % 2 == 0 else nc.sync
                    else:
                        eng = nc.sync
                    eng.dma_start(
                        out=out_view[gg],
                        in_=out4[:].rearrange("p (r c) -> p r c", r=TO),
                    )
    _split_multi_waits(nc)
    return nc


def _split_multi_waits(nc):
    """Walrus can only encode ONE sync wait per TPB instruction (the ISA
    EVENTS struct has a single wait slot); Tile sometimes attaches 2+.
    Hoist all-but-one wait into standalone EventSemaphore instructions
    placed immediately before, on the same (in-order) engine queue."""
    n_split = 0
    for fn in nc.m.functions:
        for blk in fn.blocks:
            out = []
            changed = False
            for inst in blk.instructions:
                si = inst.sync_info
                if (
                    si is not None
                    and si.on_wait
                    and len(si.on_wait) > 1
                    and not isinstance(inst, mybir.InstEventSemaphore)
                ):
                    for w in si.on_wait[:-1]:
                        ev = mybir.InstEventSemaphore(
                            name=nc.get_next_instruction_name(),
                            opcode="EventSemaphore",
                            engine=inst.engine,
                            sync_info=mybir.SyncInfo(on_wait=[w], on_update=[]),
                            bass_nofuse=True,
                        )
                        nc.inst_map[ev.name] = ev
                        out.append(ev)
                        n_split += 1
                    inst.sync_info = mybir.SyncInfo(
                        on_wait=[si.on_wait[-1]], on_update=list(si.on_update)
                    )
                    changed = True
                out.append(inst)
            if changed:
                blk.instructions = out
    return n_split


def _consts(lam, gam):
    if LAYOUT == "pair":
        ione = np.zeros((128, 130), np.float32)
        ione[:, 0:128] = np.eye(128, dtype=np.float32)
        ione[0:64, 128] = 1.0
        ione[64:128, 129] = 1.0
        ione = ione.astype(_BF16)
    else:
        ione = np.concatenate(
            [np.eye(128, dtype=np.float32), np.ones((128, 1), np.float32)], axis=1
        ).astype(_BF16)
    # lampad[q, 128j' + o] = lam[q - 64j', o] for q//64 == j', else 0.
    lampad = np.zeros((128, 2 * O), np.float32)
    lampad[0:64, 0:O] = lam
    lampad[64:128, O:2 * O] = lam
    lampad = lampad.astype(_BF16)
    # gamneg4[q, 128j + o] = -gam[q % 64, o]
    gamneg = np.concatenate([-gam, -gam], axis=0)
    gamneg4 = np.tile(gamneg, (1, 4)).astype(_BF16)
    ones128 = np.ones((128, 128), np.float32).astype(_BF16)
    return ione, lampad, gamneg4, ones128


def kernel(x, lam, gam):
    x = np.ascontiguousarray(np.asarray(x, dtype=np.float32))
    lam = np.asarray(lam, dtype=np.float32)
    gam = np.asarray(gam, dtype=np.float32)
    shard_b = x.shape[0] // N_CORES
    assert x.shape[0] % N_CORES == 0

    nc = build_nc(shard_b)
    ione, lampad, gamneg4, ones128 = _consts(lam, gam)
    in_maps = []
    for c in range(N_CORES):
        in_maps.append({
            "x": x[c * shard_b:(c + 1) * shard_b],
            "ione": ione,
            "lampad": lampad,
            "gamneg4": gamneg4,
            "ones128": ones128,
        })
    trace = bool(int(os.environ.get("KERNEL_TRACE", "0")))
    res = run_bass_kernel_spmd(
        nc, in_maps, core_ids=list(range(N_CORES)), trace=trace
    )
    LAST_RUN["exec_time_ns"] = res.exec_time_ns
    LAST_RUN["mean_exec_time_ns"] = res.mean_exec_time_ns
    out = np.concatenate([r["out"] for r in res.results], axis=0)
    return out

